# revision 21
# baseline (speedup 1.0000x reference)
"""Trainium2 Bass kernel: 3x3 stride-1 VALID conv (NHWC, HWIO) + bias + ReLU.

Problem shapes:
  x       (32, 112, 112, 64)  f32
  kernels (3, 3, 64, 128)     f32
  biases  (128,)              f32
  out     (32, 110, 110, 128) f32

Strategy:
  * Data-parallel: 4 images per core across 8 NeuronCores (no collectives).
  * Host pre-packs x into a channel/row-parity-major layout
      X[p, rp, b, w]  with p = (h%2)*64 + c,  rp = h//2
    so adjacent image rows sit on opposite halves of the 128 SBUF
    partitions. A 3x3x64 conv then becomes 6 PSUM-accumulated matmuls
    per output row (3 kw shifts x {one K=128 row-pair matmul + one K=64
    single-row matmul}), batching all 4 images into N=440 moving columns.
  * fp32r matmul dtype: full-rate (1 col/cycle) for N>=256 vs fp32's 1/4 rate.
  * ScalarE fuses bias+ReLU on the PSUM->SBUF evacuation.
  * Host restores NHWC from the [f, h, b, w] device output layout.
"""

import numpy as np

import concourse.bass as bass
import concourse.mybir as mybir
from concourse import bacc
from concourse.bass_utils import run_bass_kernel_spmd
from concourse.tile import TileContext

N_CORES = 8
B = 4  # images per core
H = W = 112
C = 64
F = 128
KH = KW = 3
HO = WO = 110
NRP = H // 2  # 56 row pairs per image
A = HO // 2  # 55 output row-parity iterations

F32 = mybir.dt.float32
F32R = mybir.dt.float32r
W_DTYPE = F32R  # walrus rejects mixing 32-bit rhs with 16-bit lhsT

X_ELEMS = NRP * B * W  # per-partition input elements (25088)
O_ELEMS = A * 2 * B * WO  # per-partition output elements (48400)

_TRACE = False
LAST_RESULT = None
_NC_CACHE = None


def _build_bass():
    nc = bacc.Bacc("TRN2", target_bir_lowering=False, debug=False)
    x_d = nc.dram_tensor("x", [128, X_ELEMS], F32R, kind="ExternalInput")
    # weights (9 stacked [128,128] lhsT tiles) + bias packed as last column
    w_d = nc.dram_tensor("w", [128, 9 * F + 1], W_DTYPE, kind="ExternalInput")
    o_d = nc.dram_tensor("o", [128, O_ELEMS], F32, kind="ExternalOutput")

    rpw = B * W  # elems per rowpair per partition (448)
    ow = 2 * B * WO  # output elems per a-iteration (880)

    with TileContext(nc) as tc:
        with (
            tc.tile_pool(name="xres", bufs=1) as xpool,
            tc.tile_pool(name="wpool", bufs=1) as wpool,
            tc.tile_pool(name="psum", bufs=8, space="PSUM") as ppool,
            tc.tile_pool(name="opool", bufs=4) as opool,
        ):
            wt = wpool.tile([128, 9 * F + 1], W_DTYPE)
            nc.sync.dma_start(out=wt[:], in_=w_d[:])
            bt = wt[:, 9 * F : 9 * F + 1].bitcast(F32)

            # Fast-start chunk schedule: small chunks first so the first
            # matmul group can begin ASAP, larger chunks once compute is
            # the slower consumer. Keep total input-side DMAs <= 8 so none
            # of them lands on a DMAHW sem lane previously used by an output
            # DMA (which would serialize input behind compute progress).
            chunk_rps = [1, 1, 2, 4, 8, 8, 32]
            assert sum(chunk_rps) == NRP
            rp2view = []  # rowpair -> (view, local index)
            for ch, nrp_ch in enumerate(chunk_rps):
                cht = xpool.tile([128, nrp_ch * rpw], F32R, tag=f"xch{ch}")
                s = len(rp2view) * rpw
                nc.sync.dma_start(out=cht[:], in_=x_d[:, s : s + nrp_ch * rpw])
                v = cht[:].rearrange("p (rp b w) -> p rp b w", rp=nrp_ch, b=B, w=W)
                for r in range(nrp_ch):
                    rp2view.append((v, r))

            def xs(lo, hi, rp, kw):
                v, r = rp2view[rp]
                return v[lo:hi, r, :, kw : kw + WO]

            wv = wt[:, 0 : 9 * F].rearrange("p (i f) -> p i f", i=9, f=F)

            for a in range(A):
                ot = opool.tile([128, ow], F32)
                for par in (0, 1):
                    ps = ppool.tile([128, B * WO], F32)
                    psv = ps[:].rearrange("p (b w) -> p b w", b=B)
                    mms = []
                    if par == 0:
                        # out row 2a: kh=0,1 -> rows 2a,2a+1 (rowpair a, K=128)
                        # with [k0;k1]; kh=2 -> row 2a+2 (low half of rowpair
                        # a+1, K=64) with k2.
                        for kw in range(KW):
                            mms.append((wv[:, kw, :], xs(0, 128, a, kw)))
                        for kw in range(KW):
                            mms.append((wv[0:64, 6 + kw, :], xs(0, 64, a + 1, kw)))
                    else:
                        # out row 2a+1: kh=0 -> row 2a+1 (high half of rowpair
                        # a, K=64) with k0; kh=1,2 -> rows 2a+2,2a+3 (rowpair
                        # a+1, K=128) with [k1;k2].
                        for kw in range(KW):
                            mms.append((wv[64:128, 6 + kw, :], xs(64, 128, a, kw)))
                        for kw in range(KW):
                            mms.append((wv[:, 3 + kw, :], xs(0, 128, a + 1, kw)))
                    for j, (lhsT, rhs) in enumerate(mms):
                        nc.tensor.matmul(
                            psv, lhsT, rhs, start=(j == 0), stop=(j == len(mms) - 1)
                        )
                    nc.scalar.activation(
                        out=ot[:, par * B * WO : (par + 1) * B * WO],
                        in_=ps[:],
                        func=mybir.ActivationFunctionType.Relu,
                        bias=bt,
                    )
                # Scalar-engine HWDGE queue: keeps output-DMA triggers (which
                # wait on ACT results) off the Sync queue so they never block
                # later input-chunk DMA triggers (head-of-line blocking).
                nc.scalar.dma_start(out=o_d[:, a * ow : (a + 1) * ow], in_=ot[:])
    nc.compile()
    return nc


def _prep_weights(kernels, biases):
    k = np.asarray(kernels, np.float32)  # (3,3,64,128) HWIO
    ws = []
    for kw in range(KW):  # [k0;k1] pairs (even rows, kh=0/1)
        ws.append(np.concatenate([k[0, kw], k[1, kw]], axis=0))
    for kw in range(KW):  # [k1;k2] pairs (odd rows, kh=1/2)
        ws.append(np.concatenate([k[1, kw], k[2, kw]], axis=0))
    for kw in range(KW):  # [k2;k0]: k2 low half (even kh=2), k0 high (odd kh=0)
        ws.append(np.concatenate([k[2, kw], k[0, kw]], axis=0))
    wdev = np.stack(ws, axis=1).reshape(128, 9 * F)
    bdev = np.asarray(biases, np.float32).reshape(128, 1)
    return np.ascontiguousarray(np.concatenate([wdev, bdev], axis=1))


def kernel(**inputs):
    global _NC_CACHE, LAST_RESULT
    x = np.asarray(inputs["x"], np.float32)
    wdev = _prep_weights(inputs["kernels"], inputs["biases"])

    if _NC_CACHE is None:
        _NC_CACHE = _build_bass()
    nc = _NC_CACHE

    in_maps = []
    for i in range(N_CORES):
        xc = x[i * B : (i + 1) * B]  # [4,112,112,64]
        # [b, rp, par, w, c] -> [par, c, rp, b, w]; partition p = par*64 + c
        xp = xc.reshape(B, NRP, 2, W, C).transpose(2, 4, 1, 0, 3)
        in_maps.append(
            {"x": np.ascontiguousarray(xp).reshape(128, X_ELEMS), "w": wdev}
        )

    LAST_RESULT = run_bass_kernel_spmd(
        nc, in_maps, core_ids=list(range(N_CORES)), trace=_TRACE
    )

    outs = []
    for res in LAST_RESULT.results:
        o = res["o"].reshape(F, A, 2, B, WO).transpose(3, 1, 2, 4, 0)
        outs.append(o.reshape(B, HO, WO, F))
    return np.ascontiguousarray(np.concatenate(outs, axis=0))


# revision 22
# speedup vs baseline: 1.0933x; 1.0933x over previous
"""Trainium2 Bass kernel: 3x3 stride-1 VALID conv (NHWC, HWIO) + bias + ReLU.

Problem shapes:
  x       (32, 112, 112, 64)  f32
  kernels (3, 3, 64, 128)     f32
  biases  (128,)              f32
  out     (32, 110, 110, 128) f32

Strategy:
  * Data-parallel: 4 images per core across 8 NeuronCores (no collectives).
  * Host pre-packs x into a channel/row-parity-major layout
      X[p, rp, b, w]  with p = (h%2)*64 + c,  rp = h//2
    so adjacent image rows sit on opposite halves of the 128 SBUF
    partitions. A 3x3x64 conv then becomes 6 PSUM-accumulated matmuls
    per output row (3 kw shifts x {one K=128 row-pair matmul + one K=64
    single-row matmul}), batching all 4 images into N=440 moving columns.
  * fp32r matmul dtype: full-rate (1 col/cycle) for N>=256 vs fp32's 1/4 rate.
  * ScalarE fuses bias+ReLU on the PSUM->SBUF evacuation.
  * Host restores NHWC from the [f, h, b, w] device output layout.
"""

import numpy as np

import concourse.bass as bass
import concourse.mybir as mybir
from concourse import bacc
from concourse.bass_utils import run_bass_kernel_spmd
from concourse.tile import TileContext

N_CORES = 8
B = 4  # images per core
H = W = 112
C = 64
F = 128
KH = KW = 3
HO = WO = 110
NRP = H // 2  # 56 row pairs per image
A = HO // 2  # 55 output row-parity iterations

F32 = mybir.dt.float32
F32R = mybir.dt.float32r
W_DTYPE = F32R  # walrus rejects mixing 32-bit rhs with 16-bit lhsT

X_ELEMS = NRP * B * W  # per-partition input elements (25088)
O_ELEMS = A * 2 * B * WO  # per-partition output elements (48400)

_TRACE = False
LAST_RESULT = None
_NC_CACHE = None


def _build_bass():
    nc = bacc.Bacc("TRN2", target_bir_lowering=False, debug=False)
    x_d = nc.dram_tensor("x", [128, X_ELEMS], F32R, kind="ExternalInput")
    # weights (9 stacked [128,128] lhsT tiles) + bias packed as last column
    w_d = nc.dram_tensor("w", [128, 9 * F + 1], W_DTYPE, kind="ExternalInput")
    o_d = nc.dram_tensor("o", [128, O_ELEMS], F32, kind="ExternalOutput")

    rpw = B * W  # elems per rowpair per partition (448)
    ow = 2 * B * WO  # output elems per a-iteration (880)

    with TileContext(nc) as tc:
        with (
            tc.tile_pool(name="xres", bufs=1) as xpool,
            tc.tile_pool(name="wpool", bufs=1) as wpool,
            tc.tile_pool(name="psum", bufs=8, space="PSUM") as ppool,
            tc.tile_pool(name="opool", bufs=4) as opool,
        ):
            wt = wpool.tile([128, 9 * F + 1], W_DTYPE)
            nc.sync.dma_start(out=wt[:], in_=w_d[:])
            bt = wt[:, 9 * F : 9 * F + 1].bitcast(F32)

            # Fast-start chunk schedule: small chunks first so the first
            # matmul group can begin ASAP, larger chunks once compute is
            # the slower consumer. Input chunks ride SWDGE (gpsimd) whose
            # DMASW sem lanes are disjoint from the DMAHW lanes used by
            # output DMAs — otherwise round-robin lane reuse makes a late
            # input chunk wait on an output DMA (head-of-line blocking).
            chunk_rps = [1, 1, 2, 4] + [8] * 6
            assert sum(chunk_rps) == NRP
            rp2view = []  # rowpair -> (view, local index)
            for ch, nrp_ch in enumerate(chunk_rps):
                cht = xpool.tile([128, nrp_ch * rpw], F32R, tag=f"xch{ch}")
                s = len(rp2view) * rpw
                nc.gpsimd.dma_start(out=cht[:], in_=x_d[:, s : s + nrp_ch * rpw])
                v = cht[:].rearrange("p (rp b w) -> p rp b w", rp=nrp_ch, b=B, w=W)
                for r in range(nrp_ch):
                    rp2view.append((v, r))

            def xs(lo, hi, rp, kw):
                v, r = rp2view[rp]
                return v[lo:hi, r, :, kw : kw + WO]

            wv = wt[:, 0 : 9 * F].rearrange("p (i f) -> p i f", i=9, f=F)

            for a in range(A):
                ot = opool.tile([128, ow], F32)
                for par in (0, 1):
                    ps = ppool.tile([128, B * WO], F32)
                    psv = ps[:].rearrange("p (b w) -> p b w", b=B)
                    mms = []
                    if par == 0:
                        # out row 2a: kh=0,1 -> rows 2a,2a+1 (rowpair a, K=128)
                        # with [k0;k1]; kh=2 -> row 2a+2 (low half of rowpair
                        # a+1, K=64) with k2.
                        for kw in range(KW):
                            mms.append((wv[:, kw, :], xs(0, 128, a, kw)))
                        for kw in range(KW):
                            mms.append((wv[0:64, 6 + kw, :], xs(0, 64, a + 1, kw)))
                    else:
                        # out row 2a+1: kh=0 -> row 2a+1 (high half of rowpair
                        # a, K=64) with k0; kh=1,2 -> rows 2a+2,2a+3 (rowpair
                        # a+1, K=128) with [k1;k2].
                        for kw in range(KW):
                            mms.append((wv[64:128, 6 + kw, :], xs(64, 128, a, kw)))
                        for kw in range(KW):
                            mms.append((wv[:, 3 + kw, :], xs(0, 128, a + 1, kw)))
                    for j, (lhsT, rhs) in enumerate(mms):
                        nc.tensor.matmul(
                            psv, lhsT, rhs, start=(j == 0), stop=(j == len(mms) - 1)
                        )
                    nc.scalar.activation(
                        out=ot[:, par * B * WO : (par + 1) * B * WO],
                        in_=ps[:],
                        func=mybir.ActivationFunctionType.Relu,
                        bias=bt,
                    )
                # Scalar-engine HWDGE queue: keeps output-DMA triggers (which
                # wait on ACT results) off the Sync queue so they never block
                # later input-chunk DMA triggers (head-of-line blocking).
                nc.scalar.dma_start(out=o_d[:, a * ow : (a + 1) * ow], in_=ot[:])
    nc.compile()
    return nc


def _prep_weights(kernels, biases):
    k = np.asarray(kernels, np.float32)  # (3,3,64,128) HWIO
    ws = []
    for kw in range(KW):  # [k0;k1] pairs (even rows, kh=0/1)
        ws.append(np.concatenate([k[0, kw], k[1, kw]], axis=0))
    for kw in range(KW):  # [k1;k2] pairs (odd rows, kh=1/2)
        ws.append(np.concatenate([k[1, kw], k[2, kw]], axis=0))
    for kw in range(KW):  # [k2;k0]: k2 low half (even kh=2), k0 high (odd kh=0)
        ws.append(np.concatenate([k[2, kw], k[0, kw]], axis=0))
    wdev = np.stack(ws, axis=1).reshape(128, 9 * F)
    bdev = np.asarray(biases, np.float32).reshape(128, 1)
    return np.ascontiguousarray(np.concatenate([wdev, bdev], axis=1))


def kernel(**inputs):
    global _NC_CACHE, LAST_RESULT
    x = np.asarray(inputs["x"], np.float32)
    wdev = _prep_weights(inputs["kernels"], inputs["biases"])

    if _NC_CACHE is None:
        _NC_CACHE = _build_bass()
    nc = _NC_CACHE

    in_maps = []
    for i in range(N_CORES):
        xc = x[i * B : (i + 1) * B]  # [4,112,112,64]
        # [b, rp, par, w, c] -> [par, c, rp, b, w]; partition p = par*64 + c
        xp = xc.reshape(B, NRP, 2, W, C).transpose(2, 4, 1, 0, 3)
        in_maps.append(
            {"x": np.ascontiguousarray(xp).reshape(128, X_ELEMS), "w": wdev}
        )

    LAST_RESULT = run_bass_kernel_spmd(
        nc, in_maps, core_ids=list(range(N_CORES)), trace=_TRACE
    )

    outs = []
    for res in LAST_RESULT.results:
        o = res["o"].reshape(F, A, 2, B, WO).transpose(3, 1, 2, 4, 0)
        outs.append(o.reshape(B, HO, WO, F))
    return np.ascontiguousarray(np.concatenate(outs, axis=0))


# revision 28
# speedup vs baseline: 1.1436x; 1.0461x over previous
"""Trainium2 Bass kernel: 3x3 stride-1 VALID conv (NHWC, HWIO) + bias + ReLU.

Problem shapes:
  x       (32, 112, 112, 64)  f32
  kernels (3, 3, 64, 128)     f32
  biases  (128,)              f32
  out     (32, 110, 110, 128) f32

Strategy:
  * Data-parallel: 4 images per core across 8 NeuronCores (no collectives).
  * Host pre-packs x into a channel/row-parity-major layout
      X[p, rp, b, w]  with p = (h%2)*64 + c,  rp = h//2
    so adjacent image rows sit on opposite halves of the 128 SBUF
    partitions. A 3x3x64 conv then becomes 6 PSUM-accumulated matmuls
    per output row (3 kw shifts x {one K=128 row-pair matmul + one K=64
    single-row matmul}), batching all 4 images into N=440 moving columns.
  * fp32r matmul dtype: full-rate (1 col/cycle) for N>=256 vs fp32's 1/4 rate.
  * ScalarE fuses bias+ReLU on the PSUM->SBUF evacuation.
  * Host restores NHWC from the [f, h, b, w] device output layout.
"""

import numpy as np

import concourse.bass as bass
import concourse.mybir as mybir
from concourse import bacc
from concourse.bass_utils import run_bass_kernel_spmd
from concourse.tile import TileContext

N_CORES = 8
B = 4  # images per core
H = W = 112
C = 64
F = 128
KH = KW = 3
HO = WO = 110
NRP = H // 2  # 56 row pairs per image
A = HO // 2  # 55 output row-parity iterations

F32 = mybir.dt.float32
F32R = mybir.dt.float32r
W_DTYPE = F32R  # walrus rejects mixing 32-bit rhs with 16-bit lhsT

X_ELEMS = NRP * B * W  # per-partition input elements (25088)
O_ELEMS = A * 2 * B * WO  # per-partition output elements (48400)

_TRACE = False
LAST_RESULT = None
_NC_CACHE = None


def _build_bass():
    nc = bacc.Bacc("TRN2", target_bir_lowering=False, debug=False)
    x_d = nc.dram_tensor("x", [128, X_ELEMS], F32R, kind="ExternalInput")
    # weights (9 stacked [128,128] lhsT tiles) + bias packed as last column
    w_d = nc.dram_tensor("w", [128, 9 * F + 1], W_DTYPE, kind="ExternalInput")
    o_d = nc.dram_tensor("o", [128, O_ELEMS], F32, kind="ExternalOutput")

    rpw = B * W  # elems per rowpair per partition (448)
    ow = 2 * B * WO  # output elems per a-iteration (880)

    with TileContext(nc) as tc:
        with (
            tc.tile_pool(name="xres", bufs=1) as xpool,
            tc.tile_pool(name="wpool", bufs=1) as wpool,
            tc.tile_pool(name="psum", bufs=8, space="PSUM") as ppool,
            tc.tile_pool(name="opool", bufs=4) as opool,
        ):
            # Scalar ring: runs concurrently with chunk 0 on SWDGE.
            wt = wpool.tile([128, 9 * F + 1], W_DTYPE)
            nc.scalar.dma_start(out=wt[:], in_=w_d[:])
            bt = wt[:, 9 * F : 9 * F + 1].bitcast(F32)

            # Fast-start chunk schedule: small chunks first so the first
            # matmul group can begin ASAP, larger chunks once compute is
            # the slower consumer. Input chunks ride SWDGE (gpsimd) whose
            # DMASW sem lanes are disjoint from the DMAHW lanes used by
            # output DMAs — otherwise round-robin lane reuse makes a late
            # input chunk wait on an output DMA (head-of-line blocking).
            chunk_rps = [1, 1, 2, 4] + [8] * 6
            assert sum(chunk_rps) == NRP
            rp2view = []  # rowpair -> (view, local index)
            for ch, nrp_ch in enumerate(chunk_rps):
                cht = xpool.tile([128, nrp_ch * rpw], F32R, tag=f"xch{ch}")
                s = len(rp2view) * rpw
                nc.gpsimd.dma_start(out=cht[:], in_=x_d[:, s : s + nrp_ch * rpw])
                v = cht[:].rearrange("p (rp b w) -> p rp b w", rp=nrp_ch, b=B, w=W)
                for r in range(nrp_ch):
                    rp2view.append((v, r))

            def xs(lo, hi, rp, kw):
                v, r = rp2view[rp]
                return v[lo:hi, r, :, kw : kw + WO]

            wv = wt[:, 0 : 9 * F].rearrange("p (i f) -> p i f", i=9, f=F)

            GRP = 2  # a-iterations per output DMA (bigger transfers, fewer DMAs)
            for a in range(A):
                ji = a % GRP
                if ji == 0:
                    n_in_g = min(GRP, A - a)
                    ot = opool.tile([128, n_in_g * ow], F32, tag="ot")
                for par in (0, 1):
                    ps = ppool.tile([128, B * WO], F32)
                    psv = ps[:].rearrange("p (b w) -> p b w", b=B)
                    mms = []
                    if par == 0:
                        # out row 2a: kh=0,1 -> rows 2a,2a+1 (rowpair a, K=128)
                        # with [k0;k1]; kh=2 -> row 2a+2 (low half of rowpair
                        # a+1, K=64) with k2.
                        for kw in range(KW):
                            mms.append((wv[:, kw, :], xs(0, 128, a, kw)))
                        for kw in range(KW):
                            mms.append((wv[0:64, 6 + kw, :], xs(0, 64, a + 1, kw)))
                    else:
                        # out row 2a+1: kh=0 -> row 2a+1 (high half of rowpair
                        # a, K=64) with k0; kh=1,2 -> rows 2a+2,2a+3 (rowpair
                        # a+1, K=128) with [k1;k2].
                        for kw in range(KW):
                            mms.append((wv[64:128, 6 + kw, :], xs(64, 128, a, kw)))
                        for kw in range(KW):
                            mms.append((wv[:, 3 + kw, :], xs(0, 128, a + 1, kw)))
                    for j, (lhsT, rhs) in enumerate(mms):
                        nc.tensor.matmul(
                            psv, lhsT, rhs, start=(j == 0), stop=(j == len(mms) - 1)
                        )
                    nc.scalar.activation(
                        out=ot[:, (ji * 2 + par) * B * WO : (ji * 2 + par + 1) * B * WO],
                        in_=ps[:],
                        func=mybir.ActivationFunctionType.Relu,
                        bias=bt,
                    )
                if ji == n_in_g - 1:
                    # Scalar-engine HWDGE queue: keeps output-DMA triggers
                    # (which wait on ACT results) off the input DMA paths.
                    g0 = a - ji
                    nc.scalar.dma_start(
                        out=o_d[:, g0 * ow : (g0 + n_in_g) * ow], in_=ot[:]
                    )
    nc.compile()
    return nc


def _prep_weights(kernels, biases):
    k = np.asarray(kernels, np.float32)  # (3,3,64,128) HWIO
    ws = []
    for kw in range(KW):  # [k0;k1] pairs (even rows, kh=0/1)
        ws.append(np.concatenate([k[0, kw], k[1, kw]], axis=0))
    for kw in range(KW):  # [k1;k2] pairs (odd rows, kh=1/2)
        ws.append(np.concatenate([k[1, kw], k[2, kw]], axis=0))
    for kw in range(KW):  # [k2;k0]: k2 low half (even kh=2), k0 high (odd kh=0)
        ws.append(np.concatenate([k[2, kw], k[0, kw]], axis=0))
    wdev = np.stack(ws, axis=1).reshape(128, 9 * F)
    bdev = np.asarray(biases, np.float32).reshape(128, 1)
    return np.ascontiguousarray(np.concatenate([wdev, bdev], axis=1))


def kernel(**inputs):
    global _NC_CACHE, LAST_RESULT
    x = np.asarray(inputs["x"], np.float32)
    wdev = _prep_weights(inputs["kernels"], inputs["biases"])

    if _NC_CACHE is None:
        _NC_CACHE = _build_bass()
    nc = _NC_CACHE

    in_maps = []
    for i in range(N_CORES):
        xc = x[i * B : (i + 1) * B]  # [4,112,112,64]
        # [b, rp, par, w, c] -> [par, c, rp, b, w]; partition p = par*64 + c
        xp = xc.reshape(B, NRP, 2, W, C).transpose(2, 4, 1, 0, 3)
        in_maps.append(
            {"x": np.ascontiguousarray(xp).reshape(128, X_ELEMS), "w": wdev}
        )

    LAST_RESULT = run_bass_kernel_spmd(
        nc, in_maps, core_ids=list(range(N_CORES)), trace=_TRACE
    )

    outs = []
    for res in LAST_RESULT.results:
        o = res["o"].reshape(F, A, 2, B, WO).transpose(3, 1, 2, 4, 0)
        outs.append(o.reshape(B, HO, WO, F))
    return np.ascontiguousarray(np.concatenate(outs, axis=0))


# revision 34
# speedup vs baseline: 1.3452x; 1.1762x over previous
"""Trainium2 Bass kernel: 3x3 stride-1 VALID conv (NHWC, HWIO) + bias + ReLU.

Problem shapes:
  x       (32, 112, 112, 64)  f32
  kernels (3, 3, 64, 128)     f32
  biases  (128,)              f32
  out     (32, 110, 110, 128) f32

Strategy:
  * Data-parallel: 4 images per core across 8 NeuronCores (no collectives).
  * Host pre-packs x into a channel/row-parity-major layout
      X[p, rp, b, w]  with p = (h%2)*64 + c,  rp = h//2
    so adjacent image rows sit on opposite halves of the 128 SBUF
    partitions. A 3x3x64 conv then becomes 6 PSUM-accumulated matmuls
    per output row (3 kw shifts x {one K=128 row-pair matmul + one K=64
    single-row matmul}), batching all 4 images into N=440 moving columns.
  * fp32r matmul dtype: full-rate (1 col/cycle) for N>=256 vs fp32's 1/4 rate.
  * ScalarE fuses bias+ReLU on the PSUM->SBUF evacuation.
  * Host restores NHWC from the [f, h, b, w] device output layout.
"""

import numpy as np

import concourse.bass as bass
import concourse.mybir as mybir
from concourse import bacc
from concourse.bass_utils import run_bass_kernel_spmd
from concourse.tile import TileContext

N_CORES = 8
B = 4  # images per core
H = W = 112
C = 64
F = 128
KH = KW = 3
HO = WO = 110
NRP = H // 2  # 56 row pairs per image
A = HO // 2  # 55 output row-parity iterations

F32 = mybir.dt.float32
F32R = mybir.dt.float32r
F16 = mybir.dt.float16
# fp16 operands: 2-byte LDWEIGHTS fast path (fp32/fp32r weights load at
# ~191ns/tile and bind the whole PE pipeline at 12 loads/row-iter), full
# 1 col/cycle stream rate, fp32 PSUM accumulation. fp16's 10 mantissa bits
# keep conv error ~3e-4 vs fp32r's 1.5e-4 (bf16 would be 3.9e-3).
MM_DTYPE = F16

X_ELEMS = NRP * B * W  # per-partition input elements (25088)
O_ELEMS = A * 2 * B * WO  # per-partition output elements (48400)

_TRACE = False
LAST_RESULT = None
_NC_CACHE = None


def _build_bass():
    nc = bacc.Bacc("TRN2", target_bir_lowering=False, debug=False)
    x_d = nc.dram_tensor("x", [128, X_ELEMS], MM_DTYPE, kind="ExternalInput")
    # weights (9 stacked [128,128] lhsT tiles) + fp32 bias packed as the
    # last two fp16 columns (bitcast back to f32 on device)
    w_d = nc.dram_tensor("w", [128, 9 * F + 2], MM_DTYPE, kind="ExternalInput")
    o_d = nc.dram_tensor("o", [128, O_ELEMS], F32, kind="ExternalOutput")

    rpw = B * W  # elems per rowpair per partition (448)
    ow = 2 * B * WO  # output elems per a-iteration (880)

    with TileContext(nc) as tc:
        with (
            tc.tile_pool(name="xres", bufs=1) as xpool,
            tc.tile_pool(name="wpool", bufs=1) as wpool,
            tc.tile_pool(name="psum", bufs=8, space="PSUM") as ppool,
            tc.tile_pool(name="opool", bufs=4) as opool,
        ):
            # Scalar ring: runs concurrently with chunk 0 on SWDGE.
            wt = wpool.tile([128, 9 * F + 2], MM_DTYPE)
            nc.scalar.dma_start(out=wt[:], in_=w_d[:])
            bt = wt[:, 9 * F : 9 * F + 2].bitcast(F32)

            # Fast-start chunk schedule: small chunks first so the first
            # matmul group can begin ASAP, larger chunks once compute is
            # the slower consumer. Input chunks ride SWDGE (gpsimd) whose
            # DMASW sem lanes are disjoint from the DMAHW lanes used by
            # output DMAs — otherwise round-robin lane reuse makes a late
            # input chunk wait on an output DMA (head-of-line blocking).
            chunk_rps = [1, 1, 2, 4] + [8] * 6
            assert sum(chunk_rps) == NRP
            rp2view = []  # rowpair -> (view, local index)
            for ch, nrp_ch in enumerate(chunk_rps):
                cht = xpool.tile([128, nrp_ch * rpw], MM_DTYPE, tag=f"xch{ch}")
                s = len(rp2view) * rpw
                nc.gpsimd.dma_start(out=cht[:], in_=x_d[:, s : s + nrp_ch * rpw])
                v = cht[:].rearrange("p (rp b w) -> p rp b w", rp=nrp_ch, b=B, w=W)
                for r in range(nrp_ch):
                    rp2view.append((v, r))

            def xs(lo, hi, rp, kw):
                v, r = rp2view[rp]
                return v[lo:hi, r, :, kw : kw + WO]

            wv = wt[:, 0 : 9 * F].rearrange("p (i f) -> p i f", i=9, f=F)

            GRP = 2  # a-iterations per output DMA (bigger transfers, fewer DMAs)
            for a in range(A):
                ji = a % GRP
                if ji == 0:
                    n_in_g = min(GRP, A - a)
                    ot = opool.tile([128, n_in_g * ow], F32, tag="ot")
                for par in (0, 1):
                    ps = ppool.tile([128, B * WO], F32)
                    psv = ps[:].rearrange("p (b w) -> p b w", b=B)
                    mms = []
                    if par == 0:
                        # out row 2a: kh=0,1 -> rows 2a,2a+1 (rowpair a, K=128)
                        # with [k0;k1]; kh=2 -> row 2a+2 (low half of rowpair
                        # a+1, K=64) with k2.
                        for kw in range(KW):
                            mms.append((wv[:, kw, :], xs(0, 128, a, kw)))
                        for kw in range(KW):
                            mms.append((wv[0:64, 6 + kw, :], xs(0, 64, a + 1, kw)))
                    else:
                        # out row 2a+1: kh=0 -> row 2a+1 (high half of rowpair
                        # a, K=64) with k0; kh=1,2 -> rows 2a+2,2a+3 (rowpair
                        # a+1, K=128) with [k1;k2].
                        for kw in range(KW):
                            mms.append((wv[64:128, 6 + kw, :], xs(64, 128, a, kw)))
                        for kw in range(KW):
                            mms.append((wv[:, 3 + kw, :], xs(0, 128, a + 1, kw)))
                    for j, (lhsT, rhs) in enumerate(mms):
                        nc.tensor.matmul(
                            psv, lhsT, rhs, start=(j == 0), stop=(j == len(mms) - 1)
                        )
                    nc.scalar.activation(
                        out=ot[:, (ji * 2 + par) * B * WO : (ji * 2 + par + 1) * B * WO],
                        in_=ps[:],
                        func=mybir.ActivationFunctionType.Relu,
                        bias=bt,
                    )
                if ji == n_in_g - 1:
                    # Scalar-engine HWDGE queue: keeps output-DMA triggers
                    # (which wait on ACT results) off the input DMA paths.
                    g0 = a - ji
                    nc.scalar.dma_start(
                        out=o_d[:, g0 * ow : (g0 + n_in_g) * ow], in_=ot[:]
                    )
    nc.compile()
    return nc


def _prep_weights(kernels, biases):
    k = np.asarray(kernels, np.float32)  # (3,3,64,128) HWIO
    ws = []
    for kw in range(KW):  # [k0;k1] pairs (even rows, kh=0/1)
        ws.append(np.concatenate([k[0, kw], k[1, kw]], axis=0))
    for kw in range(KW):  # [k1;k2] pairs (odd rows, kh=1/2)
        ws.append(np.concatenate([k[1, kw], k[2, kw]], axis=0))
    for kw in range(KW):  # [k2;k0]: k2 low half (even kh=2), k0 high (odd kh=0)
        ws.append(np.concatenate([k[2, kw], k[0, kw]], axis=0))
    wdev = np.stack(ws, axis=1).reshape(128, 9 * F).astype(np.float16)
    # fp32 bias bits carried as two fp16 columns (device bitcasts back)
    bdev = np.asarray(biases, np.float32).reshape(128, 1).view(np.float16)
    return np.ascontiguousarray(np.concatenate([wdev, bdev], axis=1))


def kernel(**inputs):
    global _NC_CACHE, LAST_RESULT
    x = np.asarray(inputs["x"], np.float32).astype(np.float16)
    wdev = _prep_weights(inputs["kernels"], inputs["biases"])

    if _NC_CACHE is None:
        _NC_CACHE = _build_bass()
    nc = _NC_CACHE

    in_maps = []
    for i in range(N_CORES):
        xc = x[i * B : (i + 1) * B]  # [4,112,112,64]
        # [b, rp, par, w, c] -> [par, c, rp, b, w]; partition p = par*64 + c
        xp = xc.reshape(B, NRP, 2, W, C).transpose(2, 4, 1, 0, 3)
        in_maps.append(
            {"x": np.ascontiguousarray(xp).reshape(128, X_ELEMS), "w": wdev}
        )

    LAST_RESULT = run_bass_kernel_spmd(
        nc, in_maps, core_ids=list(range(N_CORES)), trace=_TRACE
    )

    outs = []
    for res in LAST_RESULT.results:
        o = res["o"].reshape(F, A, 2, B, WO).transpose(3, 1, 2, 4, 0)
        outs.append(o.reshape(B, HO, WO, F))
    return np.ascontiguousarray(np.concatenate(outs, axis=0))
